# revision 4
# baseline (speedup 1.0000x reference)
"""CapsuleLayer kernel v2 — latency-optimized raw Bass (no Block).

Math (see baseline): routing stays uniform, so
  out[b, j, :] = squash(mean_n(x[b,n,:] @ W[0,n]))  for every j.

Structure (per core, 8 batch rows, data-parallel over B):
  - bf16 inputs: xt [128,72,8] (144KB), wf [128,72,16] (288KB); fp32 out
    o [128,9,8,16] (576KB). Everything tuned for makespan: measured
    NEFF epilogue is a fixed ~7.5us after the LAST instruction of any
    engine, so only the critical path to the final out-DMA completion
    matters.
  - No nc.Block (its exit barrier wastes ~0.5us).
  - Input: xt and wf interleaved per chunk in one DRAM tensor cb
    [128,72,24]; ONE fat 1728B-line transfer per HWDGE ring (sync
    cb[0:36], scalar cb[36:72]) — sub-1KB lines run ~half the ring rate.
  - 72 bf16 accumulating matmuls (PSUM pm[8,16]), gated per half.
  - squash split across engines concurrently:
      ACT:  sq = sum m^2 (Square+accum), s1 = sqrt(sq+eps)
      DVE:  md2[i,t,b*16+d] = m[b,d]*(i==b) (bf16, 2 tiles wide),
            den1 = 1+sq, u = 1/den1, fcol[i,:] = s1[i]*u[i] (bf16)
  - broadcast: two bf16 matmuls pbc_a/pbc_b[128,256] = fcol^T @ md2
    (f[b]*m[b,d] on every partition); DVE and ACT each copy one bank
    to vb4 (plain 2D, PSUM->SBUF) for 2KB-line out DMAs.
  - Output staggered per copy: sync t0-1 as soon as the ACT copy lands,
    then t2-5 after the DVE copy; scalar t6-8 (tiles identical, so any
    slab can source any vb4 offset). Sync holds the one completion wait.
"""

import os

import numpy as np

import concourse.bass as bass
import concourse.mybir as mybir
from concourse.bass_utils import run_bass_kernel_spmd

B, N, IN_DIM, OUT_DIM = 64, 1152, 8, 16
NCORES = 8
BPC = B // NCORES      # 8 batch rows per core
K = N * IN_DIM         # 9216
CK = K // 128          # 72 contraction chunks
TJ = N // 128          # 9 j-tiles
F32 = mybir.dt.float32
BF16 = mybir.dt.bfloat16
AF = mybir.ActivationFunctionType
FD = BPC * OUT_DIM     # 128 floats per j-tile per partition

_CACHE = {}
LAST_RESULT = None


def build_nc():
    nc = bass.Bass("TRN2", target_bir_lowering=False, debug=False)

    cb = nc.dram_tensor("cb", [128, CK, BPC + OUT_DIM], BF16,
                        kind="ExternalInput").ap()
    o = nc.dram_tensor("o", [128, TJ, BPC, OUT_DIM], F32, kind="ExternalOutput").ap()

    one = nc.const_aps.aps[(F32, 1.0)]

    from contextlib import ExitStack

    with ExitStack() as ctx:
        e = ctx.enter_context
        cb_t = e(nc.sbuf_tensor([128, CK * (BPC + OUT_DIM)], BF16))
        pm = e(nc.psum_tensor([BPC, OUT_DIM], F32))
        pbc_a = e(nc.psum_tensor([128, 2 * FD], F32))
        pbc_b = e(nc.psum_tensor([128, 2 * FD], F32))
        dg2 = e(nc.sbuf_tensor([BPC, 2 * FD], F32))
        ones8 = e(nc.sbuf_tensor([BPC, 128], F32))
        md2 = e(nc.sbuf_tensor([BPC, 2 * FD], BF16))
        fcol = e(nc.sbuf_tensor([BPC, 128], BF16))
        vb4 = e(nc.sbuf_tensor([128, 4 * FD], F32))
        msq = e(nc.sbuf_tensor([BPC, OUT_DIM], F32))
        sq = e(nc.sbuf_tensor([BPC, 1], F32))
        s1 = e(nc.sbuf_tensor([BPC, 1], F32))
        den1 = e(nc.sbuf_tensor([BPC, 1], F32))
        u = e(nc.sbuf_tensor([BPC, 1], F32))
        eps_t = e(nc.sbuf_tensor([128, 1], F32))
        warm = e(nc.sbuf_tensor([1, 1], F32))
        sem_a = e(nc.semaphore("sem_a"))
        sem_b = e(nc.semaphore("sem_b"))
        sem_const = e(nc.semaphore("sem_const"))
        sem_c = e(nc.semaphore("sem_c"))
        sem_sq = e(nc.semaphore("sem_sq"))
        sem_s1 = e(nc.semaphore("sem_s1"))
        sem_md = e(nc.semaphore("sem_md"))
        sem_f = e(nc.semaphore("sem_f"))
        sem_bc = e(nc.semaphore("sem_bc"))
        sem_vb = e(nc.semaphore("sem_vb"))
        sem_va = e(nc.semaphore("sem_va"))
        sem_o = e(nc.semaphore("sem_o"))
        vsem = e(nc.semaphore("vsem"))

        cb_v = cb_t.ap().rearrange("p (c z) -> p c z", z=BPC + OUT_DIM)
        vb4_v = vb4.ap().rearrange("p (t f) -> p t f", t=4)
        dg2_v = dg2.ap().rearrange("i (t b d) -> i t b d", t=2, d=OUT_DIM)
        md2_v = md2.ap().rearrange("i (t b d) -> i t b d", t=2, d=OUT_DIM)

        vcount = [0]

        def vchain(instr):
            # same-engine RAW chaining on DVE (pipeline hazard, HW-verified)
            vcount[0] += 1
            instr.then_inc(vsem, 1)
            nc.vector.wait_ge(vsem, vcount[0])
            return instr

        # ---- gpsimd: constants only ----
        nc.gpsimd.memset(eps_t.ap(), 1e-8).then_inc(sem_const, 1)
        nc.gpsimd.memset(ones8.ap(), 1.0).then_inc(sem_const, 1)
        nc.gpsimd.memset(dg2.ap(), 0.0).then_inc(sem_const, 1)
        nc.gpsimd.wait_ge(sem_const, 3)
        # dg2[i, t, b, d] = (i == b) ? 1 : 0
        nc.gpsimd.affine_select(
            out=dg2_v,
            in_=dg2_v,
            compare_op=mybir.AluOpType.not_equal,
            fill=1.0,
            base=0,
            pattern=[[0, 2], [-1, BPC], [0, OUT_DIM]],
            channel_multiplier=1,
        ).then_inc(sem_const, 1)

        # ---- sync ring: input halves 1+2; later out A + t8 ----
        nc.sync.dma_start(out=cb_v[:, 0:36, :], in_=cb[:, 0:36, :]).then_inc(
            sem_a, 16
        )

        # ---- scalar ring: input halves 3+4; squash; out B ----
        nc.scalar.dma_start(out=cb_v[:, 36:72, :], in_=cb[:, 36:72, :]).then_inc(
            sem_b, 16
        )
        # warm the Square and Sqrt tables while input flows / matmuls run
        nc.scalar.activation(warm[:, :], one[:1, :], AF.Square)
        nc.scalar.activation(warm[:, :], one[:1, :], AF.Sqrt)
        nc.scalar.wait_ge(sem_const, 1)
        # sq = sum_d m^2 straight from PSUM (one ACT op, accum output)
        nc.scalar.wait_ge(sem_c, 1)
        nc.scalar.activation(
            msq[:, :], pm[:, :], AF.Square, accum_out=sq[:, :]
        ).then_inc(sem_sq, 1)
        nc.scalar.wait_ge(sem_sq, 1)
        nc.scalar.activation(
            s1[:, :], sq[:, :], AF.Sqrt, bias=eps_t.ap()[:BPC, :]
        ).then_inc(sem_s1, 1)

        # ---- tensor: 72 accumulating matmuls + 2 broadcast matmuls ----
        for c in range(CK):
            if c == 0:
                nc.tensor.wait_ge(sem_a, 16)
            elif c == 36:
                nc.tensor.wait_ge(sem_b, 16)
            mm = nc.tensor.matmul(
                pm[:, :], cb_v[:, c, 0:BPC], cb_v[:, c, BPC:],
                start=(c == 0), stop=(c == CK - 1),
            )
        mm.then_inc(sem_c, 1)
        # pbc[p, t*128 + b*16+d] = sum_i fcol[i,p]*md2[i,...] = f[b]*m[b,d]
        nc.tensor.wait_ge(sem_f, 1)
        nc.tensor.wait_ge(sem_md, 1)
        nc.tensor.matmul(
            pbc_a[:, :], fcol.ap(), md2.ap(), start=True, stop=True
        ).then_inc(sem_bc, 1)
        nc.tensor.matmul(
            pbc_b[:, :], fcol.ap(), md2.ap(), start=True, stop=True
        ).then_inc(sem_bc, 1)

        # ---- vector: md2 + the 1/(1+sq) leg + fcol ----
        nc.vector.wait_ge(sem_c, 1)
        nc.vector.wait_ge(sem_const, 4)
        # md2[i, t, b, d] = m[b,d] * (i==b), bf16, straight from PSUM
        nc.vector.tensor_mul(
            md2_v,
            pm[:, :].unsqueeze(1).unsqueeze(1).broadcast_to(
                [BPC, 2, BPC, OUT_DIM]),
            dg2_v,
        ).then_inc(sem_md, 1)
        nc.vector.wait_ge(sem_sq, 1)
        vchain(nc.vector.tensor_scalar_add(den1[:, :], sq[:, :], 1.0))
        vchain(nc.vector.reciprocal(u[:, :], den1[:, :]))
        # fcol[i, :] = s1[i] * u[i]  (= sqrt(sq)/(1+sq) = f), bf16
        nc.vector.wait_ge(sem_s1, 1)
        nc.vector.tensor_scalar(
            fcol.ap(),
            ones8.ap(),
            s1[:, :],
            u[:, :],
            op0=mybir.AluOpType.mult,
            op1=mybir.AluOpType.mult,
        ).then_inc(sem_f, 1)
        # vb4 tiles 2-3 <- pbc_b (DVE, after mm_b); tiles 0-1 <- pbc_a
        # (ACT, can start right after mm_a)
        nc.vector.wait_ge(sem_bc, 2)
        nc.vector.tensor_copy(
            vb4.ap()[:, 2 * FD : 4 * FD], pbc_b[:, :]
        ).then_inc(sem_vb, 1)
        nc.scalar.wait_ge(sem_bc, 1)
        nc.scalar.activation(
            vb4.ap()[:, 0 : 2 * FD], pbc_a[:, :], AF.Copy
        ).then_inc(sem_va, 1)

        # ---- output: sync t0-1 (ACT tiles only) then t2-5; scalar t6-8 ----
        nc.sync.wait_ge(sem_va, 1)
        nc.sync.dma_start(
            out=o[:, 0:2, :, :], in_=vb4_v[:, 0:2, :]
        ).then_inc(sem_o, 16)
        nc.sync.wait_ge(sem_vb, 1)
        nc.sync.dma_start(
            out=o[:, 2:6, :, :], in_=vb4_v[:, 0:4, :]
        ).then_inc(sem_o, 16)
        nc.scalar.wait_ge(sem_vb, 1)
        nc.scalar.dma_start(
            out=o[:, 6:9, :, :], in_=vb4_v[:, 0:3, :]
        ).then_inc(sem_o, 16)
        # final completion wait: 3 out slabs, 16 each
        nc.sync.wait_ge(sem_o, 48)

    return nc


def _host_prep(x, W):
    import ml_dtypes

    bf16 = ml_dtypes.bfloat16
    Wf = np.asarray(W, np.float32)[0].reshape(K, OUT_DIM) * np.float32(1.0 / N)
    wf_host = Wf.reshape(CK, 128, OUT_DIM).transpose(1, 0, 2)  # [128, CK, 16]
    x = np.asarray(x, np.float32)
    in_maps = []
    for i in range(NCORES):
        xs = x[i * BPC : (i + 1) * BPC].reshape(BPC, CK, 128)
        xt_host = xs.transpose(2, 1, 0)  # [128, CK, 8]
        comb = np.concatenate([xt_host, wf_host], axis=2)  # [128, CK, 24]
        in_maps.append({"cb": np.ascontiguousarray(comb).astype(bf16)})
    return in_maps


def _unshard(results):
    out = np.empty((B, N, OUT_DIM), np.float32)
    for i in range(NCORES):
        o_np = results[i]["o"]  # [128, TJ, BPC, OUT_DIM] = (p, t, b, d)
        out[i * BPC : (i + 1) * BPC] = (
            o_np.transpose(2, 1, 0, 3).reshape(BPC, N, OUT_DIM)
        )
    return out


def kernel(x, W):
    global LAST_RESULT
    if "nc" not in _CACHE:
        _CACHE["nc"] = build_nc()
    nc = _CACHE["nc"]
    in_maps = _host_prep(x, W)
    trace = os.environ.get("KERNEL_TRACE") == "1"
    res = run_bass_kernel_spmd(nc, in_maps, list(range(NCORES)), trace=trace)
    LAST_RESULT = res
    return _unshard(res.results)
